# revision 21
# baseline (speedup 1.0000x reference)
"""Trainium2 Bass kernel for the AttentionBlock problem.

Full inputs:  x [16, 64, 64, 64] f32, w_theta [8, 64], w_phi [8, 64],
              w_g [32, 64], w_o [64, 32], gamma [] (all f32).
Sharding: data-parallel over batch, 2 samples per core on 8 NeuronCores.

Per-sample math (C=64, S=4096, T=S/4=1024):
  theta = w_theta @ x            [8, S]
  phi   = pool2x2(w_phi @ x)     [8, T]
  g     = pool2x2(w_g @ x)       [32, T]
  scoresT[t, s] = sum_c phi[c, t] theta[c, s]
  expT = exp(scoresT)            (no max-subtraction; |scores| <~ 20 is fp32-safe)
  attnU[c, s] = sum_t g[c, t] expT[t, s];  Z[s] = sum_t expT[t, s]
     (one matmul: lhsT = gT' [t, g(32) | ones(32)] so rows 32:64 of the
      output are Z broadcast across 32 partitions)
  attnS = attnU / Z
  o = (gamma * w_o) @ attnS      (gamma folded on host)
  out = o + x

Matmul operands are typed float32r (TF32-speed PE path, 4-byte layout).
"""

import sys

if "/opt/trn_rl_repo" not in sys.path:
    sys.path.insert(0, "/opt/trn_rl_repo")

import ml_dtypes
import numpy as np

import concourse.bass as bass
import concourse.tile as tile
from concourse import bacc, mybir
from concourse.bass_utils import run_bass_kernel_spmd

F32 = mybir.dt.float32
F32R = mybir.dt.float32r
BF16 = mybir.dt.bfloat16
AF = mybir.ActivationFunctionType

B, C, H, W = 16, 64, 64, 64
S = H * W            # 4096
T = S // 4           # 1024
NCORES = 8
BLOC = B // NCORES   # 2 samples per core
NT = T // 128        # 8 t-tiles
CHUNK = 1024         # s-chunk size
NCH = S // CHUNK     # 4 chunks per sample


def _phase_a(nc, tc, pools, s, x_ext, wct_sb, ident_sb, gtinit_ext):
    """Load x, fused convs, maxpools, gT setup. Returns SBUF handles."""
    (pp_sc, pp_at, pp_sm, p_samp, p_chunk) = pools

    # ---- load x: [64, 4096] -> SBUF [128, 2048]; partition p = 64*a + c
    # holds x[c, a*2048 : (a+1)*2048]
    x_sb = p_samp.tile([128, 2048], F32R, tag="x_sb")
    for q in range(4):
        nc.sync.dma_start(
            x_sb[64 * (q // 2):64 * (q // 2) + 64,
                 (q % 2) * 1024:(q % 2) * 1024 + 1024],
            x_ext[s, :, q * 1024:(q + 1) * 1024],
        )

    # ---- fused 1x1 convs: [96, 512] psum chunks -> tpg_sb [96, 4096]
    # rows 0:8 theta, 32:40 phi(unpooled), 64:96 g(unpooled) (32-aligned bases)
    tpg_sb = p_samp.tile([96, 4096], F32R, tag="tpg_sb")
    phi_sb = p_samp.tile([8, T], F32R, tag="phi_sb")
    g_sb = p_samp.tile([32, T], F32R, tag="g_sb")
    pw_sb = p_samp.tile([8, 2048], F32R, tag="pw_sb")
    gw_sb = p_samp.tile([32, 2048], F32R, tag="gw_sb")

    def pool_w(dst, src):
        # max over w-pairs: src [p, n, 2] strided view; dst [p, n]
        sv = src.rearrange("p (x two) -> p x two", two=2)
        dv = dst.rearrange("p (x one) -> p x one", one=1)
        nc.vector.tensor_max(dv, sv[:, :, 0:1], sv[:, :, 1:2])

    def pool_h(dst, src, q):
        # max over h-pairs: src [p, q, 2, 32]; dst [p, q, 32]
        sv = src.rearrange("p (q r w) -> p q r w", r=2, w=32)
        dv = dst.rearrange("p (q one w) -> p q one w", one=1, w=32)
        nc.vector.tensor_max(dv, sv[:, :, 0:1, :], sv[:, :, 1:2, :])

    # conv chunk k covers h rows 8k..8k+8 == phi/g t-tile k; pool per chunk
    # so downstream scores can start before the whole sample is done
    for k in range(8):
        a = k // 4
        ps_conv = pp_sm.tile([96, 512], F32, tag="sm", name=f"ps_conv_{s}_{k}")
        nc.tensor.matmul(
            ps_conv[:],
            wct_sb[64 * a:64 * a + 64, :],
            x_sb[64 * a:64 * a + 64, (k % 4) * 512:(k % 4) * 512 + 512],
            start=True, stop=True,
        )
        nc.vector.tensor_copy(tpg_sb[:, k * 512:(k + 1) * 512], ps_conv[:])
        pool_w(pw_sb[:, k * 256:(k + 1) * 256], tpg_sb[32:40, k * 512:(k + 1) * 512])
        pool_h(phi_sb[:, k * 128:(k + 1) * 128], pw_sb[:, k * 256:(k + 1) * 256], 4)
        pool_w(gw_sb[:, k * 256:(k + 1) * 256], tpg_sb[64:96, k * 512:(k + 1) * 512])
        pool_h(g_sb[:, k * 128:(k + 1) * 128], gw_sb[:, k * 256:(k + 1) * 256], 4)

    # ---- gT' tiles: [128, 64] per t-tile = [g block transposed | ones]
    gT_sb = p_samp.tile([128, NT * 64], BF16, tag="gT_sb")
    nc.sync.dma_start(gT_sb[:], gtinit_ext[:])
    for t in range(NT):
        ps_tr = pp_sm.tile([128, 32], F32R, tag="sm", name=f"ps_gtr_{s}_{t}")
        nc.tensor.transpose(
            ps_tr[:], g_sb[:, t * 128:(t + 1) * 128], ident_sb[0:32, 0:32]
        )
        nc.vector.tensor_copy(gT_sb[:, t * 64:t * 64 + 32], ps_tr[:])

    return x_sb, tpg_sb, phi_sb, gT_sb


def _emit_chunk(nc, pools, s, ch, handles, prev, prev2):
    """Three-stage software pipeline per emitted body:
      stage 1: scores+exp for chunk n, with chunk n-1's attn matmuls
               interleaved between t-tiles (expT(n-1) is fully ready, so
               those matmuls fill PE slack without starving ACT);
      stage 2: softmax normalization (DVE recip+mul) of chunk n-1 —
               runs on DVE while PE streams chunk n+1;
      stage 3: output conv + residual + store of chunk n-2 (its attnS is
               ready, so the o-conv never blocks the PE FIFO on DVE).
    prev  = (s, ch, expT) of chunk n-1 (needs attn+norm), or None.
    prev2 = (s, ch, attnS) of chunk n-2 (needs output), or None.
    Returns (expT(n), attnS(n-1)).
    """
    (pp_sc, pp_at, pp_sm, p_samp, p_chunk) = pools

    ps_at = None
    if prev is not None:
        ps_, ch_, expT_ = prev
        gT_sb_ = handles[ps_][3]
        ps_at = pp_at.tile([64, CHUNK], F32, tag="at",
                           name=f"ps_at_{ps_}_{ch_}")

    expT = None
    if ch is not None:
        x_sb, tpg_sb, phi_sb, gT_sb = handles[s]
        theta = tpg_sb[0:8, :]
        expT = p_chunk.tile([128, NT * CHUNK], BF16, tag="expT",
                            name=f"expT_{s}_{ch}", bufs=3)

    for t in range(NT):
        if ch is not None:
            ps_sc = pp_sc.tile([128, CHUNK], F32, tag="sc",
                               name=f"ps_sc_{s}_{ch}_{t}")
            for h in range(CHUNK // 512):
                nc.tensor.matmul(
                    ps_sc[:, h * 512:(h + 1) * 512],
                    phi_sb[:, t * 128:(t + 1) * 128],
                    theta[:, ch * CHUNK + h * 512:ch * CHUNK + (h + 1) * 512],
                    start=True, stop=True,
                )
            nc.scalar.activation(
                expT[:, t * CHUNK:(t + 1) * CHUNK], ps_sc[:], AF.Exp
            )
        if prev is not None:
            for h in range(CHUNK // 512):
                nc.tensor.matmul(
                    ps_at[:, h * 512:(h + 1) * 512],
                    gT_sb_[:, t * 64:(t + 1) * 64],
                    expT_[:, t * CHUNK + h * 512:t * CHUNK + (h + 1) * 512],
                    start=(t == 0), stop=(t == NT - 1),
                )

    attnS = None
    if prev is not None:
        rz_sb = p_chunk.tile([32, CHUNK], F32, tag="rz_sb",
                             name=f"rz_{ps_}_{ch_}")
        attnS = p_chunk.tile([32, CHUNK], F32R, tag="attnS",
                             name=f"attnS_{ps_}_{ch_}", bufs=3)
        nc.vector.reciprocal(rz_sb[:], ps_at[32:64, :])
        nc.vector.tensor_mul(attnS[:], ps_at[0:32, :], rz_sb[:])
        attnS = (ps_, ch_, attnS)

    if prev2 is not None:
        s2, ch2, attnS2 = prev2
        _emit_out(nc, pools, s2, ch2, handles[s2], attnS2)
    return expT, attnS


def _emit_out(nc, pools, s, ch, hs, attnS):
    """Output conv + residual add + store for one chunk."""
    (pp_sc, pp_at, pp_sm, p_samp, p_chunk) = pools
    x_sb = hs[0]
    out_sb = p_chunk.tile([64, CHUNK], F32, tag="out_sb", name=f"out_sb_{s}_{ch}")
    for h in range(CHUNK // 512):
        ps_o = pp_sm.tile([64, 512], F32, tag="sm", name=f"ps_o_{s}_{ch}_{h}")
        nc.tensor.matmul(
            ps_o[:], _WOG[0], attnS[:, h * 512:(h + 1) * 512],
            start=True, stop=True,
        )
        s0 = ch * CHUNK + h * 512           # global s offset
        a, b_off = s0 // 2048, s0 % 2048
        nc.vector.tensor_add(
            out_sb[:, h * 512:(h + 1) * 512],
            ps_o[:],
            x_sb[64 * a:64 * a + 64, b_off:b_off + 512].bitcast(F32),
        )
    nc.sync.dma_start(_OUT[0][s, :, ch * CHUNK:(ch + 1) * CHUNK], out_sb[:])


_WOG = [None]
_OUT = [None]


def build_nc():
    nc = bacc.Bacc("TRN2", target_bir_lowering=False, debug=False,
                   num_devices=NCORES)
    x_ext = nc.dram_tensor("x", [BLOC, C, S], F32R, kind="ExternalInput").ap()
    wct_ext = nc.dram_tensor("wct", [128, 96], F32R, kind="ExternalInput").ap()
    wog_ext = nc.dram_tensor("wog", [32, 64], F32R, kind="ExternalInput").ap()
    ident_ext = nc.dram_tensor("ident", [128, 128], F32R, kind="ExternalInput").ap()
    gtinit_ext = nc.dram_tensor("gtinit", [128, NT * 64], BF16,
                                kind="ExternalInput").ap()
    out_ext = nc.dram_tensor("out", [BLOC, C, S], F32, kind="ExternalOutput").ap()

    with tile.TileContext(nc) as tc:
        with (
            tc.tile_pool(name="wpool", bufs=1) as p_w,
            tc.tile_pool(name="samp", bufs=2) as p_samp,
            tc.tile_pool(name="chunk", bufs=2) as p_chunk,
            tc.tile_pool(name="ppsc", bufs=2, space="PSUM") as pp_sc,
            tc.tile_pool(name="ppat", bufs=1, space="PSUM") as pp_at,
            tc.tile_pool(name="ppsm", bufs=2, space="PSUM") as pp_sm,
        ):
            wct_sb = p_w.tile([128, 96], F32R, tag="wct_sb")
            wog_sb = p_w.tile([32, 64], F32R, tag="wog_sb")
            ident_sb = p_w.tile([128, 128], F32R, tag="ident_sb")
            nc.sync.dma_start(wct_sb[:], wct_ext[:])
            nc.sync.dma_start(wog_sb[:], wog_ext[:])
            nc.sync.dma_start(ident_sb[:], ident_ext[:])

            _WOG[0] = wog_sb
            _OUT[0] = out_ext
            pools = (pp_sc, pp_at, pp_sm, p_samp, p_chunk)
            handles = [
                _phase_a(nc, tc, pools, s, x_ext, wct_sb, ident_sb, gtinit_ext)
                for s in range(BLOC)
            ]
            # three-stage pipeline across the 8 (sample, chunk) pairs
            prev, prev2 = None, None
            for s in range(BLOC):
                for ch in range(NCH):
                    _, norm = _emit_chunk(nc, pools, s, ch, handles,
                                          prev, prev2)
                    expT = _
                    prev, prev2 = (s, ch, expT), norm
            _, norm = _emit_chunk(nc, pools, None, None, handles, prev, prev2)
            _emit_out(nc, pools, norm[0], norm[1], handles[norm[0]], norm[2])

    nc.compile()
    return nc


_NC_CACHE = None


def _get_nc():
    global _NC_CACHE
    if _NC_CACHE is None:
        _NC_CACHE = build_nc()
    return _NC_CACHE


def kernel(x, w_theta, w_phi, w_g, w_o, gamma):
    x = np.ascontiguousarray(np.asarray(x, dtype=np.float32))
    w_theta = np.asarray(w_theta, dtype=np.float32)
    w_phi = np.asarray(w_phi, dtype=np.float32)
    w_g = np.asarray(w_g, dtype=np.float32)
    w_o = np.asarray(w_o, dtype=np.float32)
    gamma_f = float(np.asarray(gamma, dtype=np.float32))

    # lhsT for the fused conv: [64, 96] = [w_theta.T | pad | w_phi.T | pad |
    # w_g.T] (phi at col 32, g at col 64 so SBUF partition bases stay
    # 32-aligned), replicated on partitions 64:128 (conv rhs for the second
    # x half lives at base partition 64; lhsT rows must align with rhs rows).
    wcat = np.zeros((64, 96), dtype=np.float32)
    wcat[:, 0:8] = w_theta.T
    wcat[:, 32:40] = w_phi.T
    wcat[:, 64:96] = w_g.T
    wct = np.tile(wcat, (2, 1))                        # [128, 96]
    wog = np.ascontiguousarray((gamma_f * w_o).T)      # [32, 64]
    ident = np.eye(128, dtype=np.float32)
    gtinit = np.zeros((128, NT * 64), dtype=ml_dtypes.bfloat16)
    for t in range(NT):
        gtinit[:, t * 64 + 32:t * 64 + 64] = 1.0

    nc = _get_nc()
    xr = x.reshape(B, C, S)
    in_maps = [
        {
            "x": np.ascontiguousarray(xr[i * BLOC:(i + 1) * BLOC]),
            "wct": wct,
            "wog": wog,
            "ident": ident,
            "gtinit": gtinit,
        }
        for i in range(NCORES)
    ]
    res = run_bass_kernel_spmd(nc, in_maps, core_ids=list(range(NCORES)))
    out = np.concatenate([res.results[i]["out"] for i in range(NCORES)], axis=0)
    return out.reshape(B, C, H, W).astype(np.float32)


if __name__ == "__main__":
    rng = np.random.default_rng(0)
    ins = {
        "x": rng.standard_normal((B, C, H, W), dtype=np.float32),
        "w_theta": (rng.standard_normal((8, 64)) / 8.0).astype(np.float32),
        "w_phi": (rng.standard_normal((8, 64)) / 8.0).astype(np.float32),
        "w_g": (rng.standard_normal((32, 64)) / 8.0).astype(np.float32),
        "w_o": (rng.standard_normal((64, 32)) / np.sqrt(32)).astype(np.float32),
        "gamma": np.float32(0.7),
    }
    out = kernel(**ins)
    print("out", out.shape, out.dtype, np.abs(out).mean())


# revision 25
# speedup vs baseline: 8490.2451x; 8490.2451x over previous
"""Trainium2 Bass kernel for the AttentionBlock problem.

Full inputs:  x [16, 64, 64, 64] f32, w_theta [8, 64], w_phi [8, 64],
              w_g [32, 64], w_o [64, 32], gamma [] (all f32).
Sharding: data-parallel over batch, 2 samples per core on 8 NeuronCores.

Per-sample math (C=64, S=4096, T=S/4=1024):
  theta = w_theta @ x            [8, S]
  phi   = pool2x2(w_phi @ x)     [8, T]
  g     = pool2x2(w_g @ x)       [32, T]
  scoresT[t, s] = sum_c phi[c, t] theta[c, s]
  expT = exp(scoresT)            (no max-subtraction; |scores| <~ 20 is fp32-safe)
  attnU[c, s] = sum_t g[c, t] expT[t, s];  Z[s] = sum_t expT[t, s]
     (one matmul: lhsT = gT' [t, g(32) | ones(32)] so rows 32:64 of the
      output are Z broadcast across 32 partitions)
  attnS = attnU / Z
  o = (gamma * w_o) @ attnS      (gamma folded on host)
  out = o + x

Matmul operands are typed float32r (TF32-speed PE path, 4-byte layout).
"""

import sys

if "/opt/trn_rl_repo" not in sys.path:
    sys.path.insert(0, "/opt/trn_rl_repo")

import ml_dtypes
import numpy as np

import concourse.bass as bass
import concourse.tile as tile
from concourse import bacc, mybir
from concourse.bass_utils import run_bass_kernel_spmd

F32 = mybir.dt.float32
F32R = mybir.dt.float32r
BF16 = mybir.dt.bfloat16
AF = mybir.ActivationFunctionType

B, C, H, W = 16, 64, 64, 64
S = H * W            # 4096
T = S // 4           # 1024
NCORES = 8
BLOC = B // NCORES   # 2 samples per core
NT = T // 128        # 8 t-tiles
CHUNK = 1024         # s-chunk size
NCH = S // CHUNK     # 4 chunks per sample


def _phase_a(nc, tc, pools, s, x_ext, wct_sb, ident_sb, gtinit_ext):
    """Load x, fused convs, maxpools, gT setup. Returns SBUF handles."""
    (pp_sc, pp_at, pp_sm, p_samp, p_chunk) = pools

    # ---- load x: [64, 4096] -> SBUF [128, 2048]; partition p = 64*a + c
    # holds x[c, a*2048 : (a+1)*2048]
    x_sb = p_samp.tile([128, 2048], F32R, tag="x_sb")
    for q in range(4):
        nc.sync.dma_start(
            x_sb[64 * (q // 2):64 * (q // 2) + 64,
                 (q % 2) * 1024:(q % 2) * 1024 + 1024],
            x_ext[s, :, q * 1024:(q + 1) * 1024],
        )

    # ---- fused 1x1 convs: [96, 512] psum chunks -> tpg_sb [96, 4096]
    # rows 0:8 theta, 32:40 phi(unpooled), 64:96 g(unpooled) (32-aligned bases)
    tpg_sb = p_samp.tile([96, 4096], F32R, tag="tpg_sb")
    phi_sb = p_samp.tile([8, T], F32R, tag="phi_sb")
    g_sb = p_samp.tile([32, T], F32R, tag="g_sb")
    pw_sb = p_samp.tile([8, 2048], F32R, tag="pw_sb")
    gw_sb = p_samp.tile([32, 2048], F32R, tag="gw_sb")

    def pool_w(dst, src):
        # max over w-pairs: src [p, n, 2] strided view; dst [p, n]
        sv = src.rearrange("p (x two) -> p x two", two=2)
        dv = dst.rearrange("p (x one) -> p x one", one=1)
        nc.vector.tensor_max(dv, sv[:, :, 0:1], sv[:, :, 1:2])

    def pool_h(dst, src, q):
        # max over h-pairs: src [p, q, 2, 32]; dst [p, q, 32]
        sv = src.rearrange("p (q r w) -> p q r w", r=2, w=32)
        dv = dst.rearrange("p (q one w) -> p q one w", one=1, w=32)
        nc.vector.tensor_max(dv, sv[:, :, 0:1, :], sv[:, :, 1:2, :])

    # conv chunk k covers h rows 8k..8k+8 == phi/g t-tile k; pool per chunk
    # so downstream scores can start before the whole sample is done
    for k in range(8):
        a = k // 4
        ps_conv = pp_sm.tile([96, 512], F32, tag="sm", name=f"ps_conv_{s}_{k}")
        nc.tensor.matmul(
            ps_conv[:],
            wct_sb[64 * a:64 * a + 64, :],
            x_sb[64 * a:64 * a + 64, (k % 4) * 512:(k % 4) * 512 + 512],
            start=True, stop=True,
        )
        if s == 0 and k < 2:
            nc.scalar.copy(tpg_sb[:, k * 512:(k + 1) * 512], ps_conv[:])
        else:
            nc.vector.tensor_copy(tpg_sb[:, k * 512:(k + 1) * 512], ps_conv[:])
        pool_w(pw_sb[:, k * 256:(k + 1) * 256], tpg_sb[32:40, k * 512:(k + 1) * 512])
        pool_h(phi_sb[:, k * 128:(k + 1) * 128], pw_sb[:, k * 256:(k + 1) * 256], 4)
        pool_w(gw_sb[:, k * 256:(k + 1) * 256], tpg_sb[64:96, k * 512:(k + 1) * 512])
        pool_h(g_sb[:, k * 128:(k + 1) * 128], gw_sb[:, k * 256:(k + 1) * 256], 4)

    # ---- gT' tiles: [128, 64] per t-tile = [g block transposed | ones]
    gT_sb = p_samp.tile([128, NT * 64], BF16, tag="gT_sb")
    nc.sync.dma_start(gT_sb[:], gtinit_ext[:])
    for t in range(NT):
        ps_tr = pp_sm.tile([128, 32], F32R, tag="sm", name=f"ps_gtr_{s}_{t}")
        nc.tensor.transpose(
            ps_tr[:], g_sb[:, t * 128:(t + 1) * 128], ident_sb[0:32, 0:32]
        )
        nc.vector.tensor_copy(gT_sb[:, t * 64:t * 64 + 32], ps_tr[:])

    return x_sb, tpg_sb, phi_sb, gT_sb


def _emit_chunk(nc, pools, s, ch, handles, prev, prev2, fine=False):
    """Three-stage software pipeline per emitted body:
      stage 1: scores+exp for chunk n, with chunk n-1's attn matmuls
               interleaved between t-tiles (expT(n-1) is fully ready, so
               those matmuls fill PE slack without starving ACT);
      stage 2: softmax normalization (DVE recip+mul) of chunk n-1 —
               runs on DVE while PE streams chunk n+1;
      stage 3: output conv + residual + store of chunk n-2 (its attnS is
               ready, so the o-conv never blocks the PE FIFO on DVE).
    prev  = (s, ch, expT) of chunk n-1 (needs attn+norm), or None.
    prev2 = (s, ch, attnS) of chunk n-2 (needs output), or None.
    Returns (expT(n), attnS(n-1)).
    """
    (pp_sc, pp_at, pp_sm, p_samp, p_chunk) = pools

    ps_at = None
    if prev is not None:
        ps_, ch_, expT_ = prev
        gT_sb_ = handles[ps_][3]
        ps_at = pp_at.tile([64, CHUNK], F32, tag="at",
                           name=f"ps_at_{ps_}_{ch_}")

    expT = None
    if ch is not None:
        x_sb, tpg_sb, phi_sb, gT_sb = handles[s]
        theta = tpg_sb[0:8, :]
        expT = p_chunk.tile([128, NT * CHUNK], BF16, tag="expT",
                            name=f"expT_{s}_{ch}", bufs=3)

    for t in range(NT):
        if ch is not None:
            ps_sc = pp_sc.tile([128, CHUNK], F32, tag="sc",
                               name=f"ps_sc_{s}_{ch}_{t}")
            for h in range(CHUNK // 512):
                nc.tensor.matmul(
                    ps_sc[:, h * 512:(h + 1) * 512],
                    phi_sb[:, t * 128:(t + 1) * 128],
                    theta[:, ch * CHUNK + h * 512:ch * CHUNK + (h + 1) * 512],
                    start=True, stop=True,
                )
            nc.scalar.activation(
                expT[:, t * CHUNK:(t + 1) * CHUNK], ps_sc[:], AF.Exp
            )
        if prev is not None:
            for h in range(CHUNK // 512):
                nc.tensor.matmul(
                    ps_at[:, h * 512:(h + 1) * 512],
                    gT_sb_[:, t * 64:(t + 1) * 64],
                    expT_[:, t * CHUNK + h * 512:t * CHUNK + (h + 1) * 512],
                    start=(t == 0), stop=(t == NT - 1),
                )

    attnS = None
    if prev is not None:
        rz_sb = p_chunk.tile([32, CHUNK], F32, tag="rz_sb",
                             name=f"rz_{ps_}_{ch_}")
        attnS = p_chunk.tile([32, CHUNK], F32R, tag="attnS",
                             name=f"attnS_{ps_}_{ch_}", bufs=3)
        if fine:
            # tail path: half-chunk granularity so out/DMA of h0 overlap
            # the h1 normalization
            for h in range(CHUNK // 512):
                sl = slice(h * 512, (h + 1) * 512)
                nc.vector.reciprocal(rz_sb[:, sl], ps_at[32:64, sl])
                nc.vector.tensor_mul(attnS[:, sl], ps_at[0:32, sl], rz_sb[:, sl])
                _emit_out_half(nc, pools, ps_, ch_, handles[ps_], attnS, h)
        else:
            nc.vector.reciprocal(rz_sb[:], ps_at[32:64, :])
            nc.vector.tensor_mul(attnS[:], ps_at[0:32, :], rz_sb[:])
        attnS = (ps_, ch_, attnS)

    if prev2 is not None:
        s2, ch2, attnS2 = prev2
        _emit_out(nc, pools, s2, ch2, handles[s2], attnS2)
    return expT, attnS


def _emit_out_half(nc, pools, s, ch, hs, attnS, h):
    """Output conv + residual + store for one 512-half (tail path)."""
    (pp_sc, pp_at, pp_sm, p_samp, p_chunk) = pools
    x_sb = hs[0]
    out_sb = p_chunk.tile([64, 512], F32, tag="out_h", name=f"out_h_{s}_{ch}_{h}")
    ps_o = pp_sm.tile([64, 512], F32, tag="sm", name=f"ps_of_{s}_{ch}_{h}")
    nc.tensor.matmul(
        ps_o[:], _WOG[0], attnS[:, h * 512:(h + 1) * 512],
        start=True, stop=True,
    )
    s0 = ch * CHUNK + h * 512
    a, b_off = s0 // 2048, s0 % 2048
    nc.vector.tensor_add(
        out_sb[:], ps_o[:],
        x_sb[64 * a:64 * a + 64, b_off:b_off + 512].bitcast(F32),
    )
    nc.sync.dma_start(_OUT[0][s, :, s0:s0 + 512], out_sb[:])


def _emit_out(nc, pools, s, ch, hs, attnS):
    """Output conv + residual add + store for one chunk."""
    (pp_sc, pp_at, pp_sm, p_samp, p_chunk) = pools
    x_sb = hs[0]
    out_sb = p_chunk.tile([64, CHUNK], F32, tag="out_sb", name=f"out_sb_{s}_{ch}")
    for h in range(CHUNK // 512):
        ps_o = pp_sm.tile([64, 512], F32, tag="sm", name=f"ps_o_{s}_{ch}_{h}")
        nc.tensor.matmul(
            ps_o[:], _WOG[0], attnS[:, h * 512:(h + 1) * 512],
            start=True, stop=True,
        )
        s0 = ch * CHUNK + h * 512           # global s offset
        a, b_off = s0 // 2048, s0 % 2048
        nc.vector.tensor_add(
            out_sb[:, h * 512:(h + 1) * 512],
            ps_o[:],
            x_sb[64 * a:64 * a + 64, b_off:b_off + 512].bitcast(F32),
        )
    nc.sync.dma_start(_OUT[0][s, :, ch * CHUNK:(ch + 1) * CHUNK], out_sb[:])


_WOG = [None]
_OUT = [None]


def build_nc():
    nc = bacc.Bacc("TRN2", target_bir_lowering=False, debug=False,
                   num_devices=NCORES)
    x_ext = nc.dram_tensor("x", [BLOC, C, S], F32R, kind="ExternalInput").ap()
    wct_ext = nc.dram_tensor("wct", [128, 96], F32R, kind="ExternalInput").ap()
    wog_ext = nc.dram_tensor("wog", [32, 64], F32R, kind="ExternalInput").ap()
    ident_ext = nc.dram_tensor("ident", [128, 128], F32R, kind="ExternalInput").ap()
    gtinit_ext = nc.dram_tensor("gtinit", [128, NT * 64], BF16,
                                kind="ExternalInput").ap()
    out_ext = nc.dram_tensor("out", [BLOC, C, S], F32, kind="ExternalOutput").ap()

    with tile.TileContext(nc) as tc:
        with (
            tc.tile_pool(name="wpool", bufs=1) as p_w,
            tc.tile_pool(name="samp", bufs=2) as p_samp,
            tc.tile_pool(name="chunk", bufs=2) as p_chunk,
            tc.tile_pool(name="ppsc", bufs=2, space="PSUM") as pp_sc,
            tc.tile_pool(name="ppat", bufs=1, space="PSUM") as pp_at,
            tc.tile_pool(name="ppsm", bufs=2, space="PSUM") as pp_sm,
        ):
            wct_sb = p_w.tile([128, 96], F32R, tag="wct_sb")
            wog_sb = p_w.tile([32, 64], F32R, tag="wog_sb")
            ident_sb = p_w.tile([128, 128], F32R, tag="ident_sb")
            nc.sync.dma_start(wct_sb[:], wct_ext[:])
            nc.sync.dma_start(wog_sb[:], wog_ext[:])
            nc.sync.dma_start(ident_sb[:], ident_ext[:])

            _WOG[0] = wog_sb
            _OUT[0] = out_ext
            pools = (pp_sc, pp_at, pp_sm, p_samp, p_chunk)
            handles = [None] * BLOC
            handles[0] = _phase_a(nc, tc, pools, 0, x_ext, wct_sb, ident_sb,
                                  gtinit_ext)
            # three-stage pipeline across the 8 (sample, chunk) pairs;
            # sample s+1's phase A is emitted after sample s's first chunk
            # body so its PE/DVE work fills slack instead of delaying the
            # first exps
            prev, prev2 = None, None
            for s in range(BLOC):
                for ch in range(NCH):
                    _, norm = _emit_chunk(nc, pools, s, ch, handles,
                                          prev, prev2)
                    expT = _
                    prev, prev2 = (s, ch, expT), norm
                    if ch == 0 and s + 1 < BLOC:
                        handles[s + 1] = _phase_a(nc, tc, pools, s + 1, x_ext,
                                                  wct_sb, ident_sb, gtinit_ext)
            s2, ch2, attnS2 = prev2
            _emit_out(nc, pools, s2, ch2, handles[s2], attnS2)
            _emit_chunk(nc, pools, None, None, handles, prev, None, fine=True)

    nc.compile()
    return nc


_NC_CACHE = None


def _get_nc():
    global _NC_CACHE
    if _NC_CACHE is None:
        _NC_CACHE = build_nc()
    return _NC_CACHE


def kernel(x, w_theta, w_phi, w_g, w_o, gamma):
    x = np.ascontiguousarray(np.asarray(x, dtype=np.float32))
    w_theta = np.asarray(w_theta, dtype=np.float32)
    w_phi = np.asarray(w_phi, dtype=np.float32)
    w_g = np.asarray(w_g, dtype=np.float32)
    w_o = np.asarray(w_o, dtype=np.float32)
    gamma_f = float(np.asarray(gamma, dtype=np.float32))

    # lhsT for the fused conv: [64, 96] = [w_theta.T | pad | w_phi.T | pad |
    # w_g.T] (phi at col 32, g at col 64 so SBUF partition bases stay
    # 32-aligned), replicated on partitions 64:128 (conv rhs for the second
    # x half lives at base partition 64; lhsT rows must align with rhs rows).
    wcat = np.zeros((64, 96), dtype=np.float32)
    wcat[:, 0:8] = w_theta.T
    wcat[:, 32:40] = w_phi.T
    wcat[:, 64:96] = w_g.T
    wct = np.tile(wcat, (2, 1))                        # [128, 96]
    wog = np.ascontiguousarray((gamma_f * w_o).T)      # [32, 64]
    ident = np.eye(128, dtype=np.float32)
    gtinit = np.zeros((128, NT * 64), dtype=ml_dtypes.bfloat16)
    for t in range(NT):
        gtinit[:, t * 64 + 32:t * 64 + 64] = 1.0

    nc = _get_nc()
    xr = x.reshape(B, C, S)
    in_maps = [
        {
            "x": np.ascontiguousarray(xr[i * BLOC:(i + 1) * BLOC]),
            "wct": wct,
            "wog": wog,
            "ident": ident,
            "gtinit": gtinit,
        }
        for i in range(NCORES)
    ]
    res = run_bass_kernel_spmd(nc, in_maps, core_ids=list(range(NCORES)))
    out = np.concatenate([res.results[i]["out"] for i in range(NCORES)], axis=0)
    return out.reshape(B, C, H, W).astype(np.float32)


if __name__ == "__main__":
    rng = np.random.default_rng(0)
    ins = {
        "x": rng.standard_normal((B, C, H, W), dtype=np.float32),
        "w_theta": (rng.standard_normal((8, 64)) / 8.0).astype(np.float32),
        "w_phi": (rng.standard_normal((8, 64)) / 8.0).astype(np.float32),
        "w_g": (rng.standard_normal((32, 64)) / 8.0).astype(np.float32),
        "w_o": (rng.standard_normal((64, 32)) / np.sqrt(32)).astype(np.float32),
        "gamma": np.float32(0.7),
    }
    out = kernel(**ins)
    print("out", out.shape, out.dtype, np.abs(out).mean())
